# revision 1
# baseline (speedup 1.0000x reference)
"""Trainium2 Bass kernel: 3x3 VALID conv (NCHW/OIHW) + bias + /2 + LeakyReLU.

Full-input contract: kernel(x, weight, bias) takes the complete arrays,
shards the batch dim across 8 NeuronCores (2 images per core), runs the
Bass program SPMD, and concatenates the per-core outputs.

Compute strategy (per core, per image):
  - SBUF layout: input row h, channel c -> partition 32*(h%4)+c, free
    offset (h//4)*258 + w  (rows padded 256->258 so the kw=1,2 taps can
    read a full 256-wide window without crossing rows).
  - Each output row o needs input rows o..o+2, which land in 3 distinct
    32-partition groups -> the 3 kh-taps run as concurrent 32x32 PE
    sub-tiles (tile_position row groups). 4 output rows are processed per
    round in the 4 PSUM column groups -> 12 concurrent sub-tiles.
  - kw taps are free-dim offsets (0/1/2) into the same SBUF row.
  - bf16 compute; the SWDGE input DMAs cast f32->bf16 in flight (free).
  - Each kh tap accumulates in its own PSUM plane (a region may only be
    written by one tile position); planes rotate over all 8 PSUM banks
    for eviction-chain pipelining. Eviction: ACT copy + 2 DVE adds +
    one ScalarE Lrelu (out = Lrelu(sum*0.5 + b/2), alpha=0.01) into an
    SBUF staging tile DMA'd out in 32-row batches.
"""

import sys

if "/opt/trn_rl_repo" not in sys.path:
    sys.path.insert(0, "/opt/trn_rl_repo")

import numpy as np

import concourse.bass as bass
import concourse.tile as tile
from concourse import bacc
from concourse import mybir
from concourse.bass_utils import run_bass_kernel_spmd

N_CORES = 8
IMGS_PER_CORE = 2
C = 32
H = 256
W = 256
OH = 254
OW = 254
G = 4            # partition groups = h mod 4
HD = H // G      # 64 rows per group
WPAD = W + 2     # per-row pad so kw shifts stay in-row
NFREE = 256      # matmul free dim (>=256 keeps float32r at full rate)
F32 = mybir.dt.float32
F32R = mybir.dt.float32r
BF16 = mybir.dt.bfloat16
LRELU = mybir.ActivationFunctionType.Lrelu


def build_nc(repeat=1):
    nc = bacc.Bacc()
    x_ext = nc.declare_dram_parameter(
        "x", [IMGS_PER_CORE, C, H, W], F32, isOutput=False
    )
    # host-prepared: wr[32g+k, tap, m] = weight[m, k, kh, kw]; biasr = bias/2 tiled 4x
    w_ext = nc.declare_dram_parameter("wr", [128, 9, C], BF16, isOutput=False)
    b_ext = nc.declare_dram_parameter("biasr", [128], F32, isOutput=False)
    y_ext = nc.declare_dram_parameter(
        "y", [IMGS_PER_CORE, C, OH, OW], F32, isOutput=True
    )

    with tile.TileContext(nc) as tc:
        with (
            tc.tile_pool(name="xp", bufs=2) as xpool,
            tc.tile_pool(name="const", bufs=1) as cpool,
            tc.tile_pool(name="ps", bufs=1, space="PSUM") as pspool,
            tc.tile_pool(name="ev", bufs=6) as evpool,
            tc.tile_pool(name="outp", bufs=3) as opool,
        ):
            # Weights: partition 32g+k (k = c_in), free (tap, m = c_out),
            # replicated into all 4 partition groups so lhsT.base_partition
            # matches the rhs row group (tile_position auto-derivation).
            w_sb = cpool.tile([128, 9, C], BF16)
            nc.sync.dma_start(out=w_sb, in_=w_ext[:])

            bias_half = cpool.tile([128, 1], F32)
            nc.sync.dma_start(out=bias_half, in_=b_ext[:].unsqueeze(1))


            bank_ctr = [0]
            for img_rep in range(IMGS_PER_CORE * repeat):
                img = img_rep % IMGS_PER_CORE
                x_sb = xpool.tile([128, HD, WPAD], BF16)
                nc.vector.memset(x_sb[:, :, W:WPAD], 0.0)
                # h = hd*4 + hm  ->  partition group hm, free row hd
                # SWDGE dma casts f32 -> bf16 in flight
                xsrc = x_ext[:][img].rearrange("c (hd hm) w -> hm c hd w", hm=G)
                # halves let round 0 start after ~4MB instead of 8MB
                for half in range(2):
                    hd0, hd1 = 32 * half, 32 * (half + 1)
                    for g in range(G):
                        nc.gpsimd.dma_start(
                            out=x_sb[32 * g : 32 * (g + 1), hd0:hd1, 0:W],
                            in_=xsrc[g][:, hd0:hd1, :],
                        )

                for b in range(8):  # batches of up to 32 output rows
                    rows0 = 32 * b
                    nrounds = min(8, (OH - rows0 + 3) // 4)
                    stage = opool.tile([128, 8, NFREE], F32)
                    for rb in range(nrounds):
                        h0 = rows0 + 4 * rb
                        njs = min(4, OH - h0)
                        # one PSUM plane per kh: each [32,256] region is
                        # written by exactly one PE tile position (multi-
                        # row-group accumulation into one region faults).
                        # rotate the 3 planes across all 8 PSUM banks for
                        # ~2.7 rounds of eviction-chain pipelining.
                        c0 = bank_ctr[0]
                        bank_ctr[0] += 3
                        pl0 = pspool.tile([128, NFREE], F32, tag=f"bk{c0 % 8}")
                        pl1 = pspool.tile(
                            [128, NFREE], F32, tag=f"bk{(c0 + 1) % 8}"
                        )
                        pl2 = pspool.tile(
                            [128, NFREE], F32, tag=f"bk{(c0 + 2) % 8}"
                        )
                        planes = [pl0, pl1, pl2]
                        for j in range(njs):
                            o = h0 + j
                            for kh in range(3):
                                rho = o + kh
                                g = rho % 4
                                hd = rho // 4
                                for kw in range(3):
                                    nc.tensor.matmul(
                                        planes[kh][32 * j : 32 * (j + 1), :],
                                        w_sb[
                                            32 * g : 32 * (g + 1),
                                            kh * 3 + kw,
                                            :,
                                        ],
                                        x_sb[
                                            32 * g : 32 * (g + 1),
                                            hd,
                                            kw : kw + NFREE,
                                        ],
                                        start=(kw == 0),
                                        stop=(kw == 2),
                                        tile_position=(32 * g, 32 * j),
                                    )
                        np_used = 32 * njs
                        a_sb = evpool.tile([128, NFREE], F32, tag="a")
                        a2_sb = evpool.tile([128, NFREE], F32, tag="a2")
                        b_sb = evpool.tile([128, NFREE], F32, tag="b")
                        nc.scalar.activation(
                            out=a_sb[0:np_used],
                            in_=pl0[0:np_used],
                            func=mybir.ActivationFunctionType.Copy,
                            bias=0.0,
                            scale=1.0,
                        )
                        nc.vector.tensor_add(
                            a2_sb[0:np_used], a_sb[0:np_used], pl1[0:np_used]
                        )
                        nc.vector.tensor_add(
                            b_sb[0:np_used], a2_sb[0:np_used], pl2[0:np_used]
                        )
                        nc.scalar.activation(
                            out=stage[0:np_used, rb, :],
                            in_=b_sb[0:np_used],
                            func=LRELU,
                            bias=bias_half[0:np_used],
                            scale=0.5,
                            alpha=0.01,
                        )
                    # store: per column group j, rows rows0+4*rb+j (stride 4)
                    if True:
                        for j in range(4):
                            nrb_j = 0
                            while nrb_j < nrounds and rows0 + 4 * nrb_j + j < OH:
                                nrb_j += 1
                            if nrb_j == 0:
                                continue
                            src = stage[32 * j : 32 * (j + 1), 0:nrb_j, 0:OW]
                            dst = y_ext[:][img][
                                :,
                                rows0 + j : min(rows0 + j + 4 * nrb_j, OH) : 4,
                                :,
                            ]
                            nc.sync.dma_start(out=dst, in_=src)
    nc.compile()
    return nc


_CACHE = {}


def _get_nc(repeat=1):
    key = f"nc{repeat}"
    if key not in _CACHE:
        _CACHE[key] = build_nc(repeat)
    return _CACHE[key]


def kernel(x, weight, bias):
    x = np.ascontiguousarray(np.asarray(x, dtype=np.float32))
    weight = np.asarray(weight, dtype=np.float32)
    bias = np.asarray(bias, dtype=np.float32)
    # wr[32g+k, tap, m] = weight[m, k, kh, kw], replicated into 4 groups
    import ml_dtypes
    wr = np.ascontiguousarray(
        np.tile(
            np.transpose(weight, (1, 2, 3, 0)).reshape(C, 9, C), (G, 1, 1)
        ).astype(ml_dtypes.bfloat16)
    )
    biasr = np.ascontiguousarray(np.tile(bias * 0.5, G))
    nc = _get_nc()
    in_maps = [
        {
            "x": x[IMGS_PER_CORE * i : IMGS_PER_CORE * (i + 1)],
            "wr": wr,
            "biasr": biasr,
        }
        for i in range(N_CORES)
    ]
    try:
        res = run_bass_kernel_spmd(nc, in_maps, core_ids=list(range(N_CORES)))
    except Exception:
        # transient device fault (axon terminal resets itself in ~2 min)
        import time as _time

        _time.sleep(130)
        res = run_bass_kernel_spmd(nc, in_maps, core_ids=list(range(N_CORES)))
    return np.concatenate([res.results[i]["y"] for i in range(N_CORES)], axis=0)



# revision 9
# speedup vs baseline: 5.5060x; 5.5060x over previous
"""Trainium2 Bass kernel: 3x3 VALID conv (NCHW/OIHW) + bias + /2 + LeakyReLU.

Full-input contract: kernel(x, weight, bias) takes the complete arrays,
shards the batch dim across 8 NeuronCores (2 images per core), runs the
Bass program SPMD, and concatenates the per-core outputs.

Compute strategy (per core, per image) — full-width 128-contraction:
  - Host pre-packs x to bf16 in the SBUF layout [img, 32*(h%4)+c, h//4, w]
    so input DMAs are contiguous full-128-partition transfers.
  - One "group" = 4 output rows o=4q..4q+3 needing input rows 4q..4q+5.
    Matmul A contracts all 128 partitions of slot q (rows 4q..4q+3):
    lhsT_A[(c,r),(j,m)] = w[m,c,r-j,kw], covering every (j,kh) tap with
    j+kh<=3.  Matmul B contracts slot q+1 (rows 4q+4..4q+7; bands 2,3
    carry zero weights) covering the wrap taps j+kh in {4,5}.  kw lives
    as free-dim shifts (0/1/2) with PSUM accumulation: 6 matmuls of
    [128K x 128M x 254N] per 4 output rows = 1.5 PE rows per output row
    (a 32-wide-contraction scheme needs 9).
  - PSUM: one bank holds 2 groups ([128, 2, 256] f32), 8 banks rotating.
  - Eviction: one ACT instr per bank: Lrelu(psum*0.5 + bias/2) -> bf16
    staging [128, 16, 256] (64 output rows); one DMA per staging batch.
  - y is declared in the staging layout [img, B, 32j+m, g, w]: output row
    64B+4g+j, so out-DMAs are fully contiguous.  The host descrambles,
    slices to 254x254, and casts back to f32.
  - Queues: input on SP (HWDGE), output on Pool (SWDGE), so the two
    big DMA streams and the ACT evictions all run concurrently.
"""

import sys

if "/opt/trn_rl_repo" not in sys.path:
    sys.path.insert(0, "/opt/trn_rl_repo")

import numpy as np

import concourse.bass as bass
import concourse.tile as tile
from concourse import bacc
from concourse import mybir
from concourse.bass_utils import run_bass_kernel_spmd

N_CORES = 8
IMGS_PER_CORE = 2
C = 32
H = 256
W = 256
OH = 254
OW = 254
NQ = 64          # row quads per image (slots)
NFREE = 254      # matmul free dim (= OW; kw shifts stay within the row)
F32 = mybir.dt.float32
BF16 = mybir.dt.bfloat16
LRELU = mybir.ActivationFunctionType.Lrelu


def build_nc(repeat=1):
    nc = bacc.Bacc()
    x_ext = nc.declare_dram_parameter(
        "xr", [IMGS_PER_CORE, 128, NQ, W], BF16, isOutput=False
    )
    # host-prepared (see prep_inputs): [main, wrap] lhsT weights, half-bias
    wab_ext = nc.declare_dram_parameter("wAB", [2, 128, 3, 128], BF16, isOutput=False)
    b_ext = nc.declare_dram_parameter("biasr", [128], F32, isOutput=False)
    # staging-layout output; host descrambles + slices
    y_ext = nc.declare_dram_parameter(
        "y", [IMGS_PER_CORE, 4, 128, 16, 256], BF16, isOutput=True
    )

    with tile.TileContext(nc) as tc:
        with (
            tc.tile_pool(name="xp", bufs=2) as xpool,
            tc.tile_pool(name="const", bufs=1) as cpool,
            tc.tile_pool(name="ps", bufs=1, space="PSUM") as pspool,
            tc.tile_pool(name="outp", bufs=3) as opool,
        ):
            wa_sb = cpool.tile([128, 3, 128], BF16)
            nc.sync.dma_start(out=wa_sb, in_=wa_ext[:])
            wb_sb = cpool.tile([128, 3, 128], BF16)
            nc.sync.dma_start(out=wb_sb, in_=wb_ext[:])
            bias_half = cpool.tile([128, 1], F32)
            nc.sync.dma_start(out=bias_half, in_=b_ext[:].unsqueeze(1))

            bank_ctr = [0]
            for img_rep in range(IMGS_PER_CORE * repeat):
                img = img_rep % IMGS_PER_CORE
                x_sb = xpool.tile([128, NQ, W], BF16)
                xsrc = x_ext[:][img]
                for t in range(4):  # 16-slot chunks so matmuls start early
                    nc.sync.dma_start(
                        out=x_sb[:, 16 * t : 16 * t + 16, :],
                        in_=xsrc[:, 16 * t : 16 * t + 16, :],
                    )

                for B in range(4):  # 64 output rows per staging batch
                    # cols 254-255 zeroed: keeps the out-DMA descriptors at
                    # 512B (sub-512B descriptors pay a 2x DMA penalty)
                    stage = opool.tile([128, 16, 256], BF16)
                    nc.vector.memset(stage[:, :, NFREE:256], 0.0)
                    for pp in range(8):  # bank pairs: 2 groups per bank
                        ps = pspool.tile(
                            [128, 2, 256], F32, tag=f"bk{bank_ctr[0] % 8}"
                        )
                        bank_ctr[0] += 1
                        for gg in range(2):
                            q = 16 * B + 2 * pp + gg
                            has_b = q < NQ - 1
                            for kw in range(3):
                                nc.tensor.matmul(
                                    ps[:, gg, 0:NFREE],
                                    wa_sb[:, kw, :],
                                    x_sb[:, q, kw : kw + NFREE],
                                    start=(kw == 0),
                                    stop=(kw == 2 and not has_b),
                                )
                            if has_b:
                                for kw in range(3):
                                    nc.tensor.matmul(
                                        ps[:, gg, 0:NFREE],
                                        wb_sb[:, kw, :],
                                        x_sb[:, q + 1, kw : kw + NFREE],
                                        start=False,
                                        stop=(kw == 2),
                                    )
                        nc.scalar.activation(
                            out=stage[:, 2 * pp : 2 * pp + 2, 0:NFREE],
                            in_=ps[:, :, 0:NFREE],
                            func=LRELU,
                            bias=bias_half,
                            scale=0.5,
                            alpha=0.01,
                        )
                    nc.gpsimd.dma_start(out=y_ext[:][img][B], in_=stage)
    nc.compile()
    return nc


def prep_inputs(x, weight, bias):
    """Host-side packing: per-core input dicts.

    xr[i, 32r+c, k, w] = x[i, c, 4k+r, w]  (bf16)
    wA[32r+c, kw, 32j+m] = weight[m, c, r-j, kw]   (0 <= r-j <= 2)
    wB[32r+c, kw, 32j+m] = weight[m, c, r+4-j, kw] (0 <= r+4-j <= 2)
    """
    import ml_dtypes

    x = np.asarray(x, dtype=np.float32)
    weight = np.asarray(weight, dtype=np.float32)
    bias = np.asarray(bias, dtype=np.float32)
    n = x.shape[0]
    xr = np.ascontiguousarray(
        x.reshape(n, C, NQ, 4, W)
        .transpose(0, 3, 1, 2, 4)
        .reshape(n, 128, NQ, W)
        .astype(ml_dtypes.bfloat16)
    )
    wa = np.zeros((128, 3, 128), np.float32)
    wb = np.zeros((128, 3, 128), np.float32)
    for r in range(4):
        for j in range(4):
            kh = r - j
            if 0 <= kh <= 2:
                wa[32 * r : 32 * r + 32, :, 32 * j : 32 * j + 32] = np.transpose(
                    weight[:, :, kh, :], (1, 2, 0)
                )
            kh = r + 4 - j
            if 0 <= kh <= 2:
                wb[32 * r : 32 * r + 32, :, 32 * j : 32 * j + 32] = np.transpose(
                    weight[:, :, kh, :], (1, 2, 0)
                )
    aux = {
        "wA": np.ascontiguousarray(wa.astype(ml_dtypes.bfloat16)),
        "wB": np.ascontiguousarray(wb.astype(ml_dtypes.bfloat16)),
        "biasr": np.ascontiguousarray(np.tile(bias * 0.5, 4)),
    }
    return [
        {"xr": xr[IMGS_PER_CORE * i : IMGS_PER_CORE * (i + 1)], **aux}
        for i in range(N_CORES)
    ]


def unpack_output(y):
    """y [N, 4, 128, 16, 256] staging-layout -> [N, 32, 254, 254] f32."""
    n = y.shape[0]
    out = (
        y.reshape(n, 4, 4, 32, 16, 256)  # n, B, j, m, g, w
        .transpose(0, 3, 1, 4, 2, 5)     # n, m, B, g, j, w
        .reshape(n, C, 256, 256)[:, :, :OH, :OW]
        .astype(np.float32)
    )
    return out


_CACHE = {}


def _get_nc(repeat=1):
    key = f"nc{repeat}"
    if key not in _CACHE:
        _CACHE[key] = build_nc(repeat)
    return _CACHE[key]


def kernel(x, weight, bias):
    in_maps = prep_inputs(x, weight, bias)
    nc = _get_nc()
    try:
        res = run_bass_kernel_spmd(nc, in_maps, core_ids=list(range(N_CORES)))
    except Exception:
        # transient device fault (axon terminal resets itself in ~2 min)
        import time as _time

        _time.sleep(130)
        res = run_bass_kernel_spmd(nc, in_maps, core_ids=list(range(N_CORES)))
    y = np.concatenate(
        [np.asarray(res.results[i]["y"]) for i in range(N_CORES)], axis=0
    )
    return unpack_output(y)


# revision 29
# speedup vs baseline: 7.1369x; 1.2962x over previous
"""Trainium2 Bass kernel: 3x3 VALID conv (NCHW/OIHW) + bias + /2 + LeakyReLU.

Full-input contract: kernel(x, weight, bias) takes the complete arrays,
shards the batch dim across 8 NeuronCores (2 images per core), runs the
Bass program SPMD, and concatenates the per-core outputs.

Compute strategy (per core, per image) — full-width 128-contraction,
5 matmuls per 4 output rows:
  - Host pre-packs x to bf16 in the SBUF layout [img, 32*(h%4)+c, h//4, w]
    (xr) plus an auxiliary shifted copy of the two wrap rows per slot:
    xa[img, 64s+32r2+c, k, t] = x[img, c, 4k+4+r2, t+s], s in {0,1}.
  - One "group" = 4 output rows o=4q..4q+3, consuming input rows
    4q..4q+5 x kw-shifts 0..2 = 18 (row, shift) combos.  Each 128-wide
    matmul can contract 4 combos (4 row-bands x 1 shift), so 5 matmuls:
      A(kw=0,1,2): lhsT_A[(c,r),(j,m)] = w[m,c,r-j,kw] over slot q
        covers rows 4q..4q+3 at all three shifts (12 combos).
      B1: lhsT[(s,r2,c),(j,m)] = w[m,c,4+r2-j,s] over xa slot q
        covers rows 4q+4,4q+5 at shifts 0,1 (4 combos).
      B2: same rows at shift 2 = xa's s=1 half read at offset +1
        (upper-half weights only; 2 combos).
    All five accumulate into one PSUM region with tile position (0,0).
  - PSUM: one bank holds 2 groups ([128, 2, 256] f32), 8 banks rotating.
  - Eviction: one ACT instr per bank: Lrelu(psum*0.5 + bias/2) -> bf16
    staging [128, 16, 256] (64 output rows).
  - y is declared in the staging layout [img, B, 32j+m, g, w]: output row
    64B+4g+j, fully contiguous out-DMAs; host descrambles, slices to
    254x254, casts back to f32.
  - Queues: xr on SP, xa + weights on Pool, out-DMAs alternate SP/Pool,
    evictions on ACT; all three DMA paths run concurrently in the model.
"""

import sys

if "/opt/trn_rl_repo" not in sys.path:
    sys.path.insert(0, "/opt/trn_rl_repo")

import numpy as np

import concourse.bass as bass
import concourse.tile as tile
from concourse import bacc
from concourse import mybir
from concourse.bass_utils import run_bass_kernel_spmd

N_CORES = 8
IMGS_PER_CORE = 2
C = 32
H = 256
W = 256
OH = 254
OW = 254
NQ = 64          # row quads per image (slots)
NA = 63          # aux slots (wrap rows exist for q=0..62)
NFREE = 254      # matmul free dim (= OW; kw shifts stay within the row)
F32 = mybir.dt.float32
BF16 = mybir.dt.bfloat16
LRELU = mybir.ActivationFunctionType.Lrelu


def build_nc(repeat=1):
    nc = bacc.Bacc()
    x_ext = nc.declare_dram_parameter(
        "xr", [IMGS_PER_CORE, 128, NQ, W], BF16, isOutput=False
    )
    xa_ext = nc.declare_dram_parameter(
        "xa", [IMGS_PER_CORE, 128, NA, W], BF16, isOutput=False
    )
    # host-prepared lhsT weights [kw0, kw1, kw2, B1, B2] and half-bias
    wab_ext = nc.declare_dram_parameter("wAB", [128, 5, 128], BF16, isOutput=False)
    b_ext = nc.declare_dram_parameter("biasr", [128], F32, isOutput=False)
    # staging-layout output; host descrambles + slices
    y_ext = nc.declare_dram_parameter(
        "y", [IMGS_PER_CORE, 4, 128, 16, 256], BF16, isOutput=True
    )

    with tile.TileContext(nc) as tc:
        with (
            tc.tile_pool(name="xp", bufs=2) as xpool,
            tc.tile_pool(name="xap", bufs=2) as xapool,
            tc.tile_pool(name="const", bufs=1) as cpool,
            tc.tile_pool(name="ps", bufs=1, space="PSUM") as pspool,
            tc.tile_pool(name="outp", bufs=3) as opool,
        ):
            # weights lead the SP queue (they gate the first matmul and are
            # cheaper through HWDGE); bias rides Pool between xa chunks
            wab_sb = cpool.tile([128, 5, 128], BF16)
            nc.sync.dma_start(out=wab_sb, in_=wab_ext[:])
            # bias rides the SP queue (needed only at the first eviction)
            bias_half = cpool.tile([128, 1], F32)

            bank_ctr = [0]
            for img_rep in range(IMGS_PER_CORE * repeat):
                img = img_rep % IMGS_PER_CORE
                x_sb = xpool.tile([128, NQ, W], BF16)
                xa_sb = xapool.tile([128, NA, W], BF16)
                # small first chunks so the first matmuls start ASAP
                for k0, k1 in ((0, 4), (4, 12), (12, 28), (28, 46), (46, 64)):
                    nc.sync.dma_start(
                        out=x_sb[:, k0:k1, :],
                        in_=x_ext[:][img][:, k0:k1, :],
                    )
                for k0, k1 in ((0, 4), (4, 12), (12, 28), (28, 46), (46, 63)):
                    nc.gpsimd.dma_start(
                        out=xa_sb[:, k0:k1, :],
                        in_=xa_ext[:][img][:, k0:k1, :],
                    )
                    if img_rep == 0 and k1 == 12:
                        # bias rides Pool between xa chunks (first evict
                        # only needs it at ~7us)
                        nc.gpsimd.dma_start(
                            out=bias_half, in_=b_ext[:].unsqueeze(1)
                        )

                for B in range(4):  # 64 output rows per staging batch
                    # cols 254-255 zeroed: keeps the out-DMA descriptors at
                    # 512B (sub-512B descriptors pay a 2x DMA penalty)
                    stage = opool.tile([128, 16, 256], BF16)
                    nc.vector.memset(stage[:, :, NFREE:256], 0.0)
                    for pp in range(8):  # bank pairs: 2 groups per bank
                        ps = pspool.tile(
                            [128, 2, 256], F32, tag=f"bk{bank_ctr[0] % 8}"
                        )
                        bank_ctr[0] += 1
                        for gg in range(2):
                            q = 16 * B + 2 * pp + gg
                            has_b = q < NQ - 1
                            for kw in range(3):
                                nc.tensor.matmul(
                                    ps[:, gg, 0:NFREE],
                                    wab_sb[:, kw, :],
                                    x_sb[:, q, kw : kw + NFREE],
                                    start=(kw == 0),
                                    stop=(kw == 2 and not has_b),
                                )
                            if has_b:
                                nc.tensor.matmul(
                                    ps[:, gg, 0:NFREE],
                                    wab_sb[:, 3, :],
                                    xa_sb[:, q, 0:NFREE],
                                    start=False,
                                    stop=False,
                                )
                                nc.tensor.matmul(
                                    ps[:, gg, 0:NFREE],
                                    wab_sb[:, 4, :],
                                    xa_sb[:, q, 1 : 1 + NFREE],
                                    start=False,
                                    stop=True,
                                )
                        nc.scalar.activation(
                            out=stage[:, 2 * pp : 2 * pp + 2, 0:NFREE],
                            in_=ps[:, :, 0:NFREE],
                            func=LRELU,
                            bias=bias_half,
                            scale=0.5,
                            alpha=0.01,
                        )
                        last_batch = img_rep == IMGS_PER_CORE * repeat - 1 and B == 3
                        if last_batch and pp >= 6:
                            # per-pair flush on the idle SP queue: short tail
                            s0 = 2 * pp
                            nc.sync.dma_start(
                                out=y_ext[:][img][B][:, s0 : s0 + 2, :],
                                in_=stage[:, s0 : s0 + 2, :],
                            )
                        elif pp % 2 == 1:  # flush every 2 pairs, alternate q
                            s0 = 2 * (pp - 1)
                            eng = nc.sync if pp % 4 == 1 else nc.gpsimd
                            eng.dma_start(
                                out=y_ext[:][img][B][:, s0 : s0 + 4, :],
                                in_=stage[:, s0 : s0 + 4, :],
                            )
    nc.compile()
    return nc


def prep_inputs(x, weight, bias):
    """Host-side packing: per-core input dicts.

    xr[i, 32r+c, k, w]      = x[i, c, 4k+r, w]          (bf16)
    xa[i, 64s+32r2+c, k, t] = x[i, c, 4k+4+r2, t+s]     (bf16, 0-padded)
    wAB[:, kw, :]  A taps:  [32r+c, kw, 32j+m]   = w[m,c,r-j,kw]
    wAB[:, 3, :]   B1 taps: [64s+32r2+c, 32j+m]  = w[m,c,4+r2-j,s]
    wAB[:, 4, :]   B2 taps: [64+32r2+c, 32j+m]   = w[m,c,4+r2-j,2]
    """
    import ml_dtypes

    x = np.asarray(x, dtype=np.float32)
    weight = np.asarray(weight, dtype=np.float32)
    bias = np.asarray(bias, dtype=np.float32)
    n = x.shape[0]
    xb = x.astype(ml_dtypes.bfloat16)
    xr = np.ascontiguousarray(
        xb.reshape(n, C, NQ, 4, W).transpose(0, 3, 1, 2, 4).reshape(n, 128, NQ, W)
    )
    # aux: rows 4k+4+r2 shifted left by s, s in {0,1}
    xpad = np.concatenate([xb, np.zeros((n, C, H, 1), ml_dtypes.bfloat16)], axis=3)
    xa = np.empty((n, 2, 2, C, NA, W), ml_dtypes.bfloat16)
    for s in range(2):
        for r2 in range(2):
            rows = xpad[:, :, 4 + r2 : 4 + r2 + 4 * NA : 4, s : s + W]
            xa[:, s, r2] = rows
    xa = np.ascontiguousarray(xa.transpose(0, 1, 2, 3, 4, 5).reshape(n, 128, NA, W))

    wab = np.zeros((128, 5, 128), np.float32)
    for r in range(4):
        for j in range(4):
            kh = r - j
            if 0 <= kh <= 2:
                wab[32 * r : 32 * r + 32, 0:3, 32 * j : 32 * j + 32] = np.transpose(
                    weight[:, :, kh, :], (1, 2, 0)
                )
    for s in range(2):
        for r2 in range(2):
            for j in range(4):
                kh = 4 + r2 - j
                if 0 <= kh <= 2:
                    blk = weight[:, :, kh, :]  # [m, c, kw]
                    p0 = 64 * s + 32 * r2
                    wab[p0 : p0 + 32, 3, 32 * j : 32 * j + 32] = blk[:, :, s].T
                    if s == 1:
                        wab[p0 : p0 + 32, 4, 32 * j : 32 * j + 32] = blk[:, :, 2].T
    aux = {
        "wAB": np.ascontiguousarray(wab.astype(ml_dtypes.bfloat16)),
        "biasr": np.ascontiguousarray(np.tile(bias * 0.5, 4)),
    }
    return [
        {
            "xr": xr[IMGS_PER_CORE * i : IMGS_PER_CORE * (i + 1)],
            "xa": xa[IMGS_PER_CORE * i : IMGS_PER_CORE * (i + 1)],
            **aux,
        }
        for i in range(N_CORES)
    ]


def unpack_output(y):
    """y [N, 4, 128, 16, 256] staging-layout -> [N, 32, 254, 254] f32."""
    n = y.shape[0]
    out = (
        y.reshape(n, 4, 4, 32, 16, 256)  # n, B, j, m, g, w
        .transpose(0, 3, 1, 4, 2, 5)     # n, m, B, g, j, w
        .reshape(n, C, 256, 256)[:, :, :OH, :OW]
        .astype(np.float32)
    )
    return out


_CACHE = {}


def _get_nc(repeat=1):
    key = f"nc{repeat}"
    if key not in _CACHE:
        _CACHE[key] = build_nc(repeat)
    return _CACHE[key]


def kernel(x, weight, bias):
    in_maps = prep_inputs(x, weight, bias)
    nc = _get_nc()
    try:
        res = run_bass_kernel_spmd(nc, in_maps, core_ids=list(range(N_CORES)))
    except Exception:
        # transient device fault (axon terminal resets itself in ~2 min)
        import time as _time

        _time.sleep(130)
        res = run_bass_kernel_spmd(nc, in_maps, core_ids=list(range(N_CORES)))
    y = np.concatenate(
        [np.asarray(res.results[i]["y"]) for i in range(N_CORES)], axis=0
    )
    return unpack_output(y)
